# revision 2
# baseline (speedup 1.0000x reference)
"""DiffAttention TRN2 kernel: 8-way (batch x seq-half) sharded, zero collectives.

v2 pipeline (ACT-exp is the roofline: 268M exps/core ~= 2.05ms):
  - Phase A: qkv projections to DRAM scratch (Q^T, K^T, V), token-quartered.
  - Phase B attention, combo-major ((head,branch) = combo, in 4 strip classes
    so zero-padded K tiles never need re-zeroing inside a class). S matmuls
    use kfat [128,128] lhsT = K^T rows at partition strip 32s, zeros
    elsewhere -> every phase A+B matmul runs in the same 128x128 PE tiling
    mode (no PE drain/reconfig). 3-kt S packs -> one [128,1536] PSUM tile ->
    single exp ACT (1.49us, ACT ~100% busy). Emission order S(g+1) before
    PV(g) so the in-order PE queue never stalls on the exp semaphore.
    PV lhsT = V_aug [128,65] (ones col -> softmax denominators free),
    accumulated over 32 key tiles into o_ps [65,512]; drained to bf16
    o1store/o2store (partitions 0-63) + Z rows DMA'd to base-0 z tiles.
  - Phase C tail: wide [16,2048] row math (reciprocal_approx_accurate,
    single Sqrt table switch), sel-matrix ones-matmul row broadcasts,
    mode-batched sub-loops.
  - Phase D proj: bf16 weights (host-cast), K=64 per-head contraction,
    bias added via host-tiled broadcast tensor.
"""

import sys

import numpy as np

for p in ("/opt/trn_rl_repo",):
    if p not in sys.path:
        sys.path.insert(0, p)

import ml_dtypes

import concourse.bass as bass
import concourse.bacc as bacc_mod
import concourse.mybir as mybir
from concourse.bass_utils import run_bass_kernel_spmd
from concourse.tile import TileContext

F32 = mybir.dt.float32
F32R = mybir.dt.float32r
BF16 = mybir.dt.bfloat16

B, N, DIM, H, HD = 4, 4096, 1024, 16, 32
VD = 2 * HD  # 64, per-head v dim
NQ = 2048  # query rows per core
KT = N // 128  # 32 key tiles
CIN = DIM // 128  # 8 contraction tiles
NCORES = 8
LAMBDA_INIT = 0.2
EPS = 1e-5
SCALE = HD ** -0.5

_CACHE = {}


def _r(ap):
    return ap.bitcast(F32R)


def build_nc(lam: float):
    nc = bacc_mod.Bacc(None, target_bir_lowering=False)

    xbt = nc.declare_dram_parameter("xbt", [DIM, N], F32, isOutput=False)
    wqkvt = nc.declare_dram_parameter("wqkvt", [DIM, 3 * DIM], F32, isOutput=False)
    wpbt = nc.declare_dram_parameter("wpbt", [128, CIN * DIM], BF16, isOutput=False)
    biasbc = nc.declare_dram_parameter("biasbc", [128, DIM], F32, isOutput=False)
    weff = nc.declare_dram_parameter("weff", [VD, 1], F32, isOutput=False)
    selp = nc.declare_dram_parameter("selp", [128, H * 128], F32, isOutput=False)
    y = nc.declare_dram_parameter("y", [NQ, DIM], F32, isOutput=True)

    qt_s = nc.dram_tensor("qt_scratch", [DIM, NQ], F32)
    o1_s = nc.dram_tensor("o1_scratch", [H * VD, NQ], BF16)
    u_s = nc.dram_tensor("u_scratch", [H * VD, NQ], BF16)
    kt_s = nc.dram_tensor("kt_scratch", [DIM, N], F32)
    v_s = nc.dram_tensor("v_scratch", [N, DIM], F32)

    with nc.allow_low_precision(reason="f32r bit-identical fp32; bf16 stores"), \
         TileContext(nc) as tc:
        with (
            tc.tile_pool(name="const", bufs=1) as constp,
            tc.tile_pool(name="store", bufs=1) as storep,
        ):
            # padrow0: row 0 = [1]*VD pad 0 -> broadcast partition-0 row
            padrow0 = constp.tile([128, 128], F32R)
            nc.vector.memset(padrow0.bitcast(F32), 0.0)
            nc.vector.memset(padrow0[0:1, 0:VD].bitcast(F32), 1.0)
            # padcol: col 0 = ones on partitions 0-63 -> colsum over vd
            padcol = constp.tile([128, 128], F32R)
            nc.vector.memset(padcol.bitcast(F32), 0.0)
            nc.vector.memset(padcol[0:VD, 0:1].bitcast(F32), 1.0)
            # selfat[:, h, :]: [128, 128] one-hot padded broadcast matrices
            selfat = constp.tile([128, H, 128], F32R)
            nc.sync.dma_start(
                out=selfat,
                in_=selp[:, :].rearrange("p (h v) -> p h v", v=128)
                .bitcast(F32R))
            weff_t = constp.tile([VD, 1], F32)
            nc.sync.dma_start(out=weff_t, in_=weff[:, :])

            # persistent row stores (base 0)
            z1all = storep.tile([H, NQ], F32)
            argall = storep.tile([H, NQ], F32)

            if True:
                # ================= phase A: qkv =================
                with (
                    tc.tile_pool(name="xbt_p", bufs=2) as xbtp,
                    tc.tile_pool(name="wq_p", bufs=4) as wqp,
                    tc.tile_pool(name="wv_p", bufs=2) as wvp,
                    tc.tile_pool(name="drain_p", bufs=3) as drp,
                    tc.tile_pool(name="psA", bufs=3, space="PSUM") as psA,
                    tc.tile_pool(name="psAv", bufs=2, space="PSUM") as psAv,
                ):
                    for tq in range(4):  # token quarters of 1024
                        xb = xbtp.tile([128, CIN, 1024], F32R, tag="xb")
                        nc.sync.dma_start(
                            out=xb,
                            in_=xbt[:, tq * 1024:(tq + 1) * 1024]
                            .rearrange("(t p) n -> p t n", p=128).bitcast(F32R),
                        )
                        for co in range(2 * CIN):  # 0..7 Q, 8..15 K
                            is_q = co < CIN
                            if is_q and tq >= 2:
                                continue
                            ps = psA.tile([128, 1024], F32, tag="ps")
                            for ci in range(CIN):
                                wt = wqp.tile([128, 128], F32R, tag="w")
                                nc.sync.dma_start(
                                    out=wt,
                                    in_=wqkvt[ci * 128:(ci + 1) * 128,
                                              co * 128:(co + 1) * 128]
                                    .bitcast(F32R),
                                )
                                for sb in range(2):
                                    nc.tensor.matmul(
                                        ps[:, sb * 512:(sb + 1) * 512],
                                        _r(wt),
                                        _r(xb[:, ci, sb * 512:(sb + 1) * 512]),
                                        start=(ci == 0),
                                        stop=(ci == CIN - 1),
                                    )
                            dr = drp.tile([128, 1024], F32, tag="dr")
                            nc.vector.tensor_copy(dr, ps)
                            dst = qt_s if is_q else kt_s
                            coo = co if is_q else co - CIN
                            nc.sync.dma_start(
                                out=dst[coo * 128:(coo + 1) * 128,
                                        tq * 1024:(tq + 1) * 1024],
                                in_=dr,
                            )
                        for cc in range(DIM // 512):
                            wv = wvp.tile([128, CIN, 512], F32R, tag="wv")
                            nc.sync.dma_start(
                                out=wv,
                                in_=wqkvt[:, 2 * DIM + cc * 512:
                                          2 * DIM + (cc + 1) * 512]
                                .rearrange("(t p) n -> p t n", p=128)
                                .bitcast(F32R),
                            )
                            for kt in range(8):
                                psv = psAv.tile([128, 512], F32, tag="psv")
                                for ci in range(CIN):
                                    nc.tensor.matmul(
                                        psv,
                                        _r(xb[:, ci, kt * 128:(kt + 1) * 128]),
                                        _r(wv[:, ci, :]),
                                        start=(ci == 0),
                                        stop=(ci == CIN - 1),
                                    )
                                drv = drp.tile([128, 512], F32, tag="drv")
                                if kt % 2 == 0:
                                    nc.vector.tensor_copy(drv, psv)
                                else:
                                    nc.scalar.activation(
                                        drv, psv,
                                        mybir.ActivationFunctionType.Copy)
                                nc.sync.dma_start(
                                    out=v_s[tq * 1024 + kt * 128:
                                            tq * 1024 + (kt + 1) * 128,
                                            cc * 512:(cc + 1) * 512],
                                    in_=drv,
                                )

                # ================= phase B: attention =================
                with (
                    tc.tile_pool(name="kfat_p", bufs=1) as kfatp,
                    tc.tile_pool(name="vh_p", bufs=1) as vhp,
                    tc.tile_pool(name="qp_p", bufs=1) as qpp,
                    tc.tile_pool(name="exp_p", bufs=4) as expp,
                    tc.tile_pool(name="stag_p", bufs=2) as stagp,
                    tc.tile_pool(name="hsc_p", bufs=2) as hscp,
                    tc.tile_pool(name="psS", bufs=2, space="PSUM") as psS,
                    tc.tile_pool(name="psO", bufs=2, space="PSUM") as psO,
                ):
                    kfats = [kfatp.tile([128, KT, 128], F32R, tag=f"k{i}", name=f"kfat{i}")
                             for i in range(2)]
                    vhs = [vhp.tile([128, KT, 65], F32R, tag=f"v{i}", name=f"vh{i}")
                           for i in range(2)]
                    qps = [qpp.tile([128, NQ], F32R, tag=f"q{i}", name=f"qp{i}")
                           for i in range(2)]
                    trowfs = [hscp.tile([128, 512], F32R, tag=f"tr{i}",
                                        name=f"trowf{i}") for i in range(2)]
                    u2fs = [hscp.tile([128, 512], F32R, tag=f"u2{i}",
                                      name=f"u2f{i}") for i in range(2)]
                    for i in range(2):
                        nc.vector.memset(trowfs[i].bitcast(F32), 0.0)
                        nc.vector.memset(u2fs[i][VD:128, :].bitcast(F32), 0.0)
                    for i in range(2):
                        nc.vector.memset(kfats[i].bitcast(F32), 0.0)
                        nc.vector.memset(vhs[i][:, :, VD:65].bitcast(F32), 1.0)

                    groups = [list(range(3 * g, 3 * g + 3)) for g in range(10)]
                    groups.append([30, 31])
                    packs = []
                    for cidx in range(32):
                        for qb in range(4):
                            for gi, g in enumerate(groups):
                                packs.append(
                                    (cidx, qb, g, gi == 0, gi == len(groups) - 1))

                    st = {"o": {}, "ex": {}}
                    sched = {}

                    def combo_hbr(idx):
                        s, pc = idx // 8, idx % 8
                        return 2 * pc + (s >> 1), s & 1, s

                    def emit_setup(idx):
                        h, br, s = combo_hbr(idx)
                        kf, vh, qp = kfats[idx % 2], vhs[idx % 2], qps[idx % 2]
                        if idx in (8, 16, 24) and idx % 2 == 0:
                            pass
                        if idx in (8, 16, 24):
                            s_old = (idx - 1) // 8
                            for i in range(2):
                                nc.vector.memset(
                                    kfats[i][32 * s_old:32 * s_old + 32, :, :]
                                    .bitcast(F32), 0.0)
                        r0 = h * VD + br * HD
                        nc.sync.dma_start(
                            out=kf[32 * s:32 * s + 32, :, :],
                            in_=kt_s[r0:r0 + HD, :]
                            .rearrange("p (k t) -> p k t", t=128).bitcast(F32R),
                        )
                        nc.sync.dma_start(
                            out=vh[:, :, 0:VD],
                            in_=v_s[:, h * VD:(h + 1) * VD]
                            .rearrange("(k p) v -> p k v", p=128).bitcast(F32R),
                        )
                        hp = h // 2
                        nc.sync.dma_start(
                            out=qp,
                            in_=qt_s[hp * 128:(hp + 1) * 128, :].bitcast(F32R),
                        )

                    def emit_S(pi):
                        cidx, qb, g, first, last = packs[pi]
                        if first and qb == 0:
                            emit_setup(cidx)
                        kf, qp = kfats[cidx % 2], qps[cidx % 2]
                        n = len(g)
                        sps = psS.tile([128, 1536], F32, tag="s")
                        for i, kt in enumerate(g):
                            nc.tensor.matmul(
                                sps[:, i * 512:(i + 1) * 512],
                                _r(kf[:, kt, :]),
                                _r(qp[:, qb * 512:(qb + 1) * 512]),
                                start=True, stop=True,
                            )
                        ex = expp.tile([128, 1536], F32R, tag="e")
                        nc.scalar.activation(
                            ex[:, 0:n * 512], sps[:, 0:n * 512],
                            mybir.ActivationFunctionType.Exp, scale=SCALE,
                        )
                        st["ex"][pi] = ex

                    def emit_PV(pi):
                        cidx, qb, g, first, last = packs[pi]

                        h, br, s = combo_hbr(cidx)
                        vh = vhs[cidx % 2]
                        ex = st["ex"].pop(pi)
                        if first:
                            st["o"][(cidx, qb)] = psO.tile([65, 512], F32, tag="o", name="o_ps")
                        o_ps = st["o"][(cidx, qb)]
                        for i, kt in enumerate(g):
                            nc.tensor.matmul(
                                o_ps,
                                _r(vh[:, kt, :]),
                                ex[:, i * 512:(i + 1) * 512],
                                start=(kt == 0),
                                stop=(kt == KT - 1),
                            )
                        if last and br == 0:
                            o_ps = st["o"].pop((cidx, qb))
                            sl = slice(qb * 512, (qb + 1) * 512)
                            stag_o = stagp.tile([VD, 512], BF16, tag="so")
                            nc.vector.tensor_copy(stag_o, o_ps[0:VD, :])
                            nc.sync.dma_start(
                                out=o1_s[h * VD:(h + 1) * VD, sl], in_=stag_o)
                            stag_z = stagp.tile([65, 512], F32, tag="sz")
                            nc.vector.tensor_copy(
                                stag_z[VD:65, :], o_ps[VD:65, :])
                            nc.sync.dma_start(
                                out=z1all[h:h + 1, sl], in_=stag_z[VD:65, :])
                        elif last:
                            # br1: hoisted combine, staggered so the PE-queue
                            # matmuls never starve the exp stream
                            o_ps = st["o"].pop((cidx, qb))
                            sl = slice(qb * 512, (qb + 1) * 512)
                            rsl = slice(h * VD, (h + 1) * VD)
                            nh = st["nh"] = st.get("nh", -1) + 1
                            trowf, u2f = trowfs[nh % 2], u2fs[nh % 2]
                            box = {}

                            def hoist_a(h=h, sl=sl, rsl=rsl, o_ps=o_ps,
                                        trowf=trowf, box=box):
                                zp0 = hscp.tile([1, 512], F32, tag="zp0",
                                                name="zp0")
                                nc.sync.dma_start(
                                    out=zp0, in_=z1all[h:h + 1, sl])
                                o1t = hscp.tile([VD, 512], BF16, tag="o1t",
                                                name="o1t")
                                nc.sync.dma_start(out=o1t, in_=o1_s[rsl, sl])
                                z2c = hscp.tile([65, 512], F32, tag="z2c",
                                                name="z2c")
                                nc.vector.tensor_copy(
                                    z2c[VD:65, :], o_ps[VD:65, :])
                                z2p0 = hscp.tile([1, 512], F32, tag="z2p0",
                                                 name="z2p0")
                                nc.sync.dma_start(
                                    out=z2p0, in_=z2c[VD:65, :])
                                rz2 = hscp.tile([1, 512], F32, tag="rz2",
                                                name="rz2")
                                rzs = hscp.tile([1, 512], F32, tag="rzs",
                                                name="rzs")
                                nc.vector.reciprocal_approx_accurate(
                                    out=rz2, in_=z2p0, scratch=rzs)
                                nc.vector.scalar_tensor_tensor(
                                    out=trowf[0:1, :], in0=zp0,
                                    scalar=float(lam), in1=rz2,
                                    op0=mybir.AluOpType.mult,
                                    op1=mybir.AluOpType.mult)
                                tbc = psO.tile([128, 512], F32, tag="o",
                                               name="tbc")
                                nc.tensor.matmul(
                                    tbc, padrow0, trowf,
                                    start=True, stop=True)
                                box["tbc"] = tbc
                                box["o1t"] = o1t
                                box["zp0"] = zp0

                            def hoist_b(h=h, sl=sl, rsl=rsl, o_ps=o_ps,
                                        u2f=u2f, box=box):
                                tbc, o1t, zp0 = (box["tbc"], box["o1t"],
                                                 box["zp0"])
                                o2s = hscp.tile([VD, 512], F32, tag="o2s",
                                                name="o2s")
                                nc.vector.tensor_copy(o2s, o_ps[0:VD, :])
                                o2x = hscp.tile([VD, 512], F32, tag="o2x",
                                                name="o2x")
                                nc.vector.tensor_mul(o2x, o2s, tbc[0:VD, :])
                                ut = hscp.tile([VD, 512], BF16, tag="ut",
                                               name="ut")
                                nc.vector.tensor_sub(ut, o1t, o2x)
                                nc.sync.dma_start(out=u_s[rsl, sl], in_=ut)
                                nc.vector.tensor_mul(u2f[0:VD, :], ut, ut)
                                mps = psO.tile([128, 512], F32, tag="o",
                                               name="mps")
                                nc.tensor.matmul(
                                    mps, padcol, u2f, start=True, stop=True)
                                ze = hscp.tile([1, 512], F32, tag="ze",
                                               name="ze")
                                nc.vector.tensor_scalar_mul(
                                    ze, zp0, float(EPS ** 0.5))
                                zsq = hscp.tile([1, 512], F32, tag="zsq",
                                                name="zsq")
                                nc.vector.tensor_mul(zsq, ze, ze)
                                arg0 = hscp.tile([1, 512], F32, tag="arg0",
                                                 name="arg0")
                                nc.vector.scalar_tensor_tensor(
                                    out=arg0, in0=mps[0:1, :],
                                    scalar=1.0 / VD, in1=zsq,
                                    op0=mybir.AluOpType.mult,
                                    op1=mybir.AluOpType.add)
                                nc.sync.dma_start(
                                    out=argall[h:h + 1, sl], in_=arg0)

                            sched.setdefault(pi + 3, []).append(hoist_a)
                            sched.setdefault(pi + 6, []).append(hoist_b)

                    emit_S(0)
                    for pi in range(len(packs) - 1):
                        emit_S(pi + 1)
                        emit_PV(pi)
                        for fn in sched.pop(pi, []):
                            fn()
                    emit_PV(len(packs) - 1)
                    for kk in sorted(sched):
                        for fn in sched[kk]:
                            fn()

                # ============ phase C: tail (norm + proj) ============
                with tc.tile_pool(name="row_p", bufs=1) as rowp:
                    rr0f = rowp.tile([128, NQ], F32R, tag="rr0f")
                    nc.vector.memset(rr0f.bitcast(F32), 0.0)
                    with tc.tile_pool(name="rowW", bufs=3) as roww:
                        sd = roww.tile([H, NQ], F32, tag="t")
                        nc.scalar.activation(
                            sd, argall, mybir.ActivationFunctionType.Sqrt)
                        rrt = roww.tile([H, NQ], F32, tag="t")
                        scr2 = roww.tile([H, NQ], F32, tag="t")
                        nc.vector.reciprocal_approx_accurate(
                            out=rrt, in_=sd, scratch=scr2)
                        nc.vector.tensor_copy(rr0f[0:H, :], rrt)

                    # o_n = u * rr * weff -> onstore [128, CIN, NQ] bf16
                    onstore = rowp.tile([128, CIN, NQ], BF16, tag="onstore")
                    with (
                        tc.tile_pool(name="inN", bufs=4) as inp,
                        tc.tile_pool(name="scrN", bufs=4) as scrp,
                        tc.tile_pool(name="psN", bufs=2, space="PSUM") as psN,
                    ):
                        for h in range(H):
                            for qb in range(4):
                                sl = slice(qb * 512, (qb + 1) * 512)
                                rsl = slice(h * VD, (h + 1) * VD)
                                ut = inp.tile([VD, 512], BF16, tag="ut")
                                nc.sync.dma_start(out=ut, in_=u_s[rsl, sl])
                                rrbc = psN.tile([128, 512], F32, tag="rrbc")
                                nc.tensor.matmul(
                                    rrbc, selfat[:, h, :], rr0f[:, sl],
                                    start=True, stop=True)
                                on = scrp.tile([VD, 512], F32, tag="on")
                                nc.vector.tensor_mul(on, ut, rrbc[0:VD, :])
                                if h % 2 == 0:
                                    nc.vector.tensor_scalar_mul(
                                        onstore[0:VD, h // 2, sl], on, weff_t)
                                else:
                                    onb = scrp.tile([VD, 512], BF16, tag="onb")
                                    nc.vector.tensor_scalar_mul(
                                        onb, on, weff_t)
                                    nc.sync.dma_start(
                                        out=onstore[VD:128, h // 2, sl],
                                        in_=onb)

                    # ============ phase D: proj (bf16, K=128) ============
                    with (
                        tc.tile_pool(name="wp_p", bufs=1) as wpp,
                        tc.tile_pool(name="yd_p", bufs=3) as ydp,
                        tc.tile_pool(name="psY", bufs=2, space="PSUM") as psY,
                    ):
                        wpb = wpp.tile([128, CIN, DIM], BF16)
                        nc.sync.dma_start(
                            out=wpb,
                            in_=wpbt[:, :].rearrange("v (c n) -> v c n", c=CIN))
                        bb = wpp.tile([128, DIM], F32)
                        nc.sync.dma_start(out=bb, in_=biasbc[:, :])
                        for qt in range(NQ // 128):
                            yps = psY.tile([128, 1024], F32, tag="y")
                            for sb in range(2):
                                for ci in range(CIN):
                                    nc.tensor.matmul(
                                        yps[:, sb * 512:(sb + 1) * 512],
                                        onstore[:, ci, qt * 128:(qt + 1) * 128],
                                        wpb[:, ci, sb * 512:(sb + 1) * 512],
                                        start=(ci == 0),
                                        stop=(ci == CIN - 1),
                                    )
                            yd = ydp.tile([128, 1024], F32, tag="yd")
                            nc.vector.tensor_add(yd, yps, bb)
                            nc.sync.dma_start(
                                out=y[qt * 128:(qt + 1) * 128, :], in_=yd)
    nc.finalize()
    return nc


def _make_inputs(x, w_qkv, w_proj, b_proj, sub_norm_w):
    wqkvt = np.ascontiguousarray(np.asarray(w_qkv, np.float32).T)
    wprojt = np.ascontiguousarray(np.asarray(w_proj, np.float32).T)  # [c, out]
    # proj weights: partition (h%2)*64+vd, col (h//2)*DIM+out
    wpbt = np.ascontiguousarray(
        wprojt.reshape(CIN, 2, VD, DIM).transpose(1, 2, 0, 3)
        .reshape(128, CIN * DIM)).astype(ml_dtypes.bfloat16)
    biasbc = np.ascontiguousarray(
        np.tile(np.asarray(b_proj, np.float32).reshape(1, DIM), (128, 1)))
    # selfat[:, h, :]: [128,128]; rows 0-15 hold one-hot h -> cols 0-63
    selp = np.zeros((128, H, 128), np.float32)
    for h in range(H):
        selp[h, h, 0:VD] = 1.0
    selp = np.ascontiguousarray(selp.reshape(128, H * 128))
    weff = (np.asarray(sub_norm_w, np.float32)
            * (1.0 - LAMBDA_INIT)).reshape(VD, 1)
    return wqkvt, wpbt, biasbc, weff, selp


def _in_maps(inputs):
    x = np.asarray(inputs["x"], np.float32)
    wqkvt, wpbt, biasbc, weff, selp = _make_inputs(
        x, inputs["w_qkv"], inputs["w_proj"], inputs["b_proj"],
        inputs["sub_norm_w"])
    in_maps = []
    for c in range(NCORES):
        b, half = c // 2, c % 2
        xt = np.asarray(x[b].T)  # [DIM, N]
        if half == 1:  # query rows first
            xt = np.concatenate([xt[:, NQ:], xt[:, :NQ]], axis=1)
        in_maps.append({
            "xbt": np.ascontiguousarray(xt),
            "wqkvt": wqkvt,
            "wpbt": wpbt,
            "biasbc": biasbc,
            "weff": weff,
            "selp": selp,
        })
    return in_maps


def kernel(x, w_qkv, w_proj, b_proj, lambda_q1, lambda_k1, lambda_q2,
           lambda_k2, sub_norm_w):
    lam = float(
        np.exp(np.sum(np.float64(lambda_q1) * np.float64(lambda_k1)))
        - np.exp(np.sum(np.float64(lambda_q2) * np.float64(lambda_k2)))
        + LAMBDA_INIT
    )

    key = round(lam, 12)
    if key not in _CACHE:
        _CACHE[key] = build_nc(lam)
    nc = _CACHE[key]

    in_maps = _in_maps(dict(
        x=x, w_qkv=w_qkv, w_proj=w_proj, b_proj=b_proj, sub_norm_w=sub_norm_w))
    res = run_bass_kernel_spmd(nc, in_maps, list(range(NCORES)))
    out = np.empty((B, N, DIM), np.float32)
    for c in range(NCORES):
        b, half = c // 2, c % 2
        out[b, half * NQ:(half + 1) * NQ, :] = res.results[c]["y"]
    return out



# revision 16
# speedup vs baseline: 1.1636x; 1.1636x over previous
"""DiffAttention TRN2 kernel v3: 8-way (batch x seq-half) sharded, zero collectives.

v3 pipeline (ACT-exp roofline ~2.15ms/core; v2 was PE-bound at 3.03ms):
  - All matmul operands bf16 (fp32 runs 2-pass fp32_mode=HIGH at ~2x cost).
  - Phase A: qkv projections -> DRAM scratch (Q^T, K^T bf16, V bf16).
  - Phase B pair-centric: pair = (head h, branches 0/1). Q^T/K^T 128-row
    blocks naturally hold 4 combos in 4 32-partition strips; S matmuls are
    K=32 row-tiled (tile_position=(32s,0)) so strip MMs run concurrently.
    3-slot groups -> S into 3 PSUM banks -> one [128,1536] exp ACT (bf16
    out), ping-pong 3+3 banks; PV accumulates per-combo [65,512] (65th
    row = ones col of V_aug -> softmax denominators) in 2 more banks.
    Combine (u = o1 - lam*z1/z2*o2, arg = mean(u^2)+eps*z1^2) uses DVE rows
    + GPSIMD partition_broadcast/all_reduce; u kept in SBUF (ustore).
  - Phase C: rsqrt rows + sel-matmul broadcast -> onstore bf16.
  - Phase D: proj bf16, K=128 per chunk, bias via host-tiled broadcast.
"""

import sys

import numpy as np

for p in ("/opt/trn_rl_repo",):
    if p not in sys.path:
        sys.path.insert(0, p)

import ml_dtypes

import concourse.bass as bass
import concourse.bacc as bacc_mod
import concourse.bass_isa as bass_isa
import concourse.mybir as mybir
from concourse.bass_utils import run_bass_kernel_spmd
from concourse.tile import TileContext

F32 = mybir.dt.float32
F32R = mybir.dt.float32r
BF16 = mybir.dt.bfloat16

B, N, DIM, H, HD = 4, 4096, 1024, 16, 32
VD = 2 * HD  # 64, per-head v dim
NQ = 2048  # query rows per core
KT = N // 128  # 32 key tiles
CIN = DIM // 128  # 8 contraction tiles
NCORES = 8
LAMBDA_INIT = 0.2
EPS = 1e-5
SCALE = HD ** -0.5

_CACHE = {}


def build_nc(lam: float):
    nc = bacc_mod.Bacc(None, target_bir_lowering=False)

    xbt = nc.declare_dram_parameter("xbt", [DIM, N], F32, isOutput=False)
    wqkvt = nc.declare_dram_parameter("wqkvt", [DIM, 3 * DIM], BF16,
                                      isOutput=False)
    wpbt = nc.declare_dram_parameter("wpbt", [128, CIN * DIM], BF16,
                                     isOutput=False)
    biasbc = nc.declare_dram_parameter("biasbc", [128, DIM], F32,
                                       isOutput=False)
    weff = nc.declare_dram_parameter("weff", [128, 1], F32, isOutput=False)
    selp = nc.declare_dram_parameter("selp", [128, H * 128], F32,
                                     isOutput=False)
    y = nc.declare_dram_parameter("y", [NQ, DIM], F32, isOutput=True)

    qt_s = nc.dram_tensor("qt_scratch", [DIM, NQ], BF16)
    kt_s = nc.dram_tensor("kt_scratch", [DIM, N], BF16)
    v_s = nc.dram_tensor("v_scratch", [N, DIM], BF16)

    with nc.allow_low_precision(reason="bf16 matmul operands; bf16 stores"), \
         TileContext(nc) as tc:
        with (
            tc.tile_pool(name="const", bufs=1) as constp,
            tc.tile_pool(name="store", bufs=1) as storep,
        ):
            # selfat[:, h, :]: [128, 128] one-hot padded broadcast matrices
            selfat = constp.tile([128, H, 128], F32R)
            nc.sync.dma_start(
                out=selfat,
                in_=selp[:, :].rearrange("p (h v) -> p h v", v=128)
                .bitcast(F32R))
            weff_t = constp.tile([128, 1], F32)
            nc.sync.dma_start(out=weff_t, in_=weff[:, :])

            # persistent stores
            argall = storep.tile([H, NQ], F32)
            ustore = storep.tile([128, CIN, NQ], BF16)

            # ================= phase A: qkv (bf16) =================
            with (
                tc.tile_pool(name="xbt_p", bufs=2) as xbtp,
                tc.tile_pool(name="wq_p", bufs=4) as wqp,
                tc.tile_pool(name="wv_p", bufs=2) as wvp,
                tc.tile_pool(name="drain_p", bufs=3) as drp,
                tc.tile_pool(name="psA", bufs=3, space="PSUM") as psA,
                tc.tile_pool(name="psAv", bufs=2, space="PSUM") as psAv,
            ):
                for tq in range(4):  # token quarters of 1024
                    xbf = xbtp.tile([128, CIN, 1024], F32, tag="xbf")
                    nc.sync.dma_start(
                        out=xbf,
                        in_=xbt[:, tq * 1024:(tq + 1) * 1024]
                        .rearrange("(t p) n -> p t n", p=128),
                    )
                    xb = xbtp.tile([128, CIN, 1024], BF16, tag="xb")
                    nc.vector.tensor_copy(xb, xbf)
                    for co in range(2 * CIN):  # 0..7 Q, 8..15 K
                        is_q = co < CIN
                        if is_q and tq >= 2:
                            continue
                        ps = psA.tile([128, 1024], F32, tag="ps")
                        for ci in range(CIN):
                            wt = wqp.tile([128, 128], BF16, tag="w")
                            nc.sync.dma_start(
                                out=wt,
                                in_=wqkvt[ci * 128:(ci + 1) * 128,
                                          co * 128:(co + 1) * 128],
                            )
                            for sb in range(2):
                                nc.tensor.matmul(
                                    ps[:, sb * 512:(sb + 1) * 512],
                                    wt,
                                    xb[:, ci, sb * 512:(sb + 1) * 512],
                                    start=(ci == 0),
                                    stop=(ci == CIN - 1),
                                )
                        dr = drp.tile([128, 1024], BF16, tag="dr")
                        nc.vector.tensor_copy(dr, ps)
                        dst = qt_s if is_q else kt_s
                        coo = co if is_q else co - CIN
                        nc.sync.dma_start(
                            out=dst[coo * 128:(coo + 1) * 128,
                                    tq * 1024:(tq + 1) * 1024],
                            in_=dr,
                        )
                    for cc in range(DIM // 512):
                        wv = wvp.tile([128, CIN, 512], BF16, tag="wv")
                        nc.sync.dma_start(
                            out=wv,
                            in_=wqkvt[:, 2 * DIM + cc * 512:
                                      2 * DIM + (cc + 1) * 512]
                            .rearrange("(t p) n -> p t n", p=128),
                        )
                        for kt in range(8):
                            psv = psAv.tile([128, 512], F32, tag="psv")
                            for ci in range(CIN):
                                nc.tensor.matmul(
                                    psv,
                                    xb[:, ci, kt * 128:(kt + 1) * 128],
                                    wv[:, ci, :],
                                    start=(ci == 0),
                                    stop=(ci == CIN - 1),
                                )
                            drv = drp.tile([128, 512], BF16, tag="drv")
                            if kt % 2 == 0:
                                nc.vector.tensor_copy(drv, psv)
                            else:
                                nc.scalar.activation(
                                    drv, psv,
                                    mybir.ActivationFunctionType.Copy)
                            nc.sync.dma_start(
                                out=v_s[tq * 1024 + kt * 128:
                                        tq * 1024 + (kt + 1) * 128,
                                        cc * 512:(cc + 1) * 512],
                                in_=drv,
                            )

            # ================= phase B: attention =================
            # slots per (head h, qb): [(br, kt) for kt for br] -> groups of 3
            slots_hq = [(br, kt) for kt in range(KT) for br in (0, 1)]
            GPQ = (len(slots_hq) + 2) // 3  # 22 groups (last has 1 slot)
            groups_hq = [slots_hq[3 * i:3 * i + 3] for i in range(GPQ)]

            with (
                tc.tile_pool(name="kf_p", bufs=2) as kfp,
                tc.tile_pool(name="qp_p", bufs=2) as qpp,
                tc.tile_pool(name="vh_p", bufs=2) as vhp,
                tc.tile_pool(name="exp_p", bufs=4) as expp,
                tc.tile_pool(name="cp_p", bufs=3) as cpp,
                tc.tile_pool(name="row_p", bufs=3) as rowp,
                tc.tile_pool(name="psS", bufs=2, space="PSUM") as psS,
                tc.tile_pool(name="psO", bufs=2, space="PSUM") as psO,
            ):
                st = {}
                sched = {}

                def emit_setup_hp(hp):
                    kf = kfp.tile([128, KT, 128], BF16, tag="kf")
                    nc.sync.dma_start(
                        out=kf,
                        in_=kt_s[hp * 128:(hp + 1) * 128, :]
                        .rearrange("p (k t) -> p k t", t=128),
                    )
                    qp = qpp.tile([128, NQ], BF16, tag="qp")
                    nc.sync.dma_start(
                        out=qp, in_=qt_s[hp * 128:(hp + 1) * 128, :])
                    st[("kf", hp)] = kf
                    st[("qp", hp)] = qp
                    st.pop(("kf", hp - 2), None)
                    st.pop(("qp", hp - 2), None)

                def emit_setup_h(h):
                    vh = vhp.tile([128, KT, 65], BF16, tag="vh")
                    nc.sync.dma_start(
                        out=vh[:, :, 0:VD],
                        in_=v_s[:, h * VD:(h + 1) * VD]
                        .rearrange("(k p) v -> p k v", p=128),
                    )
                    nc.vector.memset(vh[:, :, VD:65], 1.0)
                    st[("vh", h)] = vh
                    st.pop(("vh", h - 2), None)

                def emit_S(h, qb, g, gidx):
                    if g == 0 and qb == 0:
                        if h % 2 == 0:
                            emit_setup_hp(h // 2)
                        emit_setup_h(h)
                    kf, qp = st[("kf", h // 2)], st[("qp", h // 2)]
                    sps = psS.tile([128, 3, 512], F32, tag="s")
                    for j, (br, kt) in enumerate(groups_hq[g]):
                        s = 2 * (h % 2) + br
                        nc.tensor.matmul(
                            sps[:, j, :],
                            kf[32 * s:32 * s + 32, kt, :],
                            qp[32 * s:32 * s + 32,
                               qb * 512:(qb + 1) * 512],
                            start=True, stop=True,
                            tile_position=(32 * s, 0),
                        )
                    n = len(groups_hq[g])
                    ex = expp.tile([128, 3, 512], BF16, tag="e")
                    nc.scalar.activation(
                        ex[:, 0:n, :], sps[:, 0:n, :],
                        mybir.ActivationFunctionType.Exp, scale=SCALE,
                    )
                    st[("ex", gidx)] = ex

                def emit_PV(h, qb, g, gidx):
                    vh = st[("vh", h)]
                    ex = st.pop(("ex", gidx))
                    for j, (br, kt) in enumerate(groups_hq[g]):
                        if kt == 0:
                            st[("o", br)] = psO.tile(
                                [65, 512], F32, tag="o", name=f"o{br}")
                        nc.tensor.matmul(
                            st[("o", br)],
                            vh[:, kt, 0:65],
                            ex[:, j, :],
                            start=(kt == 0),
                            stop=(kt == KT - 1),
                        )
                    if g == GPQ - 1:
                        emit_drain(h, qb, gidx)

                def emit_drain(h, qb, gidx):
                    # full-tile copies free the 2 PV banks ASAP
                    o1p = st.pop(("o", 0))
                    o2p = st.pop(("o", 1))
                    sl = slice(qb * 512, (qb + 1) * 512)
                    cp_o1 = cpp.tile([65, 512], F32, tag="co1", name="co1")
                    nc.vector.tensor_copy(cp_o1, o1p)
                    cp_o2 = cpp.tile([65, 512], F32, tag="co2", name="co2")
                    nc.vector.tensor_copy(cp_o2, o2p)
                    # z rows to partition 0 (DVE is lane-locked; DMA moves)
                    z1 = rowp.tile([1, 512], F32, tag="z1", name="z1")
                    nc.sync.dma_start(out=z1, in_=cp_o1[VD:65, :])
                    z2 = rowp.tile([1, 512], F32, tag="z2", name="z2")
                    nc.sync.dma_start(out=z2, in_=cp_o2[VD:65, :])

                    def part1(h=h, sl=sl, cp_o1=cp_o1, cp_o2=cp_o2,
                              z1=z1, z2=z2):
                        rzs = rowp.tile([1, 512], F32, tag="rzs", name="rzs")
                        rz2 = rowp.tile([1, 512], F32, tag="rz2", name="rz2")
                        nc.vector.reciprocal_approx_accurate(
                            out=rz2, in_=z2, scratch=rzs)
                        trow = rowp.tile([1, 512], F32, tag="tr", name="tr")
                        nc.vector.scalar_tensor_tensor(
                            out=trow, in0=z1, scalar=float(lam), in1=rz2,
                            op0=mybir.AluOpType.mult,
                            op1=mybir.AluOpType.mult)
                        tbc = cpp.tile([VD, 512], F32, tag="tbc", name="tbc")
                        nc.gpsimd.partition_broadcast(tbc, trow)
                        st[("t", h, sl.start)] = (tbc, cp_o1, cp_o2, z1)

                    def part2(h=h, qb=qb, sl=sl):
                        tbc, cp_o1, cp_o2, z1 = st.pop(("t", h, sl.start))
                        x1 = cpp.tile([VD, 512], F32, tag="x1", name="x1")
                        nc.vector.tensor_mul(x1, cp_o2[0:VD, :], tbc)
                        u_t = cpp.tile([VD, 512], BF16, tag="ut", name="ut")
                        nc.vector.tensor_sub(u_t, cp_o1[0:VD, :], x1)
                        rsl = slice((h % 2) * VD, (h % 2) * VD + VD)
                        nc.sync.dma_start(
                            out=ustore[rsl, h // 2, sl], in_=u_t)
                        u2 = cpp.tile([VD, 512], F32, tag="u2", name="u2")
                        nc.vector.tensor_mul(u2, u_t, u_t)
                        s2 = cpp.tile([VD, 512], F32, tag="s2", name="s2")
                        nc.gpsimd.partition_all_reduce(
                            s2, u2, channels=VD,
                            reduce_op=bass_isa.ReduceOp.add)
                        st[("u", h, sl.start)] = (s2, z1)

                    def part3(h=h, sl=sl):
                        s2, z1 = st.pop(("u", h, sl.start))
                        ze = rowp.tile([1, 512], F32, tag="ze", name="ze")
                        nc.vector.tensor_scalar_mul(
                            ze, z1, float(EPS ** 0.5))
                        zsq = rowp.tile([1, 512], F32, tag="zq", name="zq")
                        nc.vector.tensor_mul(zsq, ze, ze)
                        arg0 = rowp.tile([1, 512], F32, tag="ar", name="ar")
                        nc.vector.scalar_tensor_tensor(
                            out=arg0, in0=s2[0:1, :],
                            scalar=1.0 / VD, in1=zsq,
                            op0=mybir.AluOpType.mult,
                            op1=mybir.AluOpType.add)
                        nc.sync.dma_start(out=argall[h:h + 1, sl], in_=arg0)

                    sched.setdefault(gidx + 2, []).append(part1)
                    sched.setdefault(gidx + 5, []).append(part2)
                    sched.setdefault(gidx + 8, []).append(part3)

                items = [(h, qb, g)
                         for h in range(H)
                         for qb in range(4)
                         for g in range(GPQ)]
                LOOK = 2
                for j in range(LOOK):
                    emit_S(*items[j], j)
                for gidx in range(len(items)):
                    if gidx + LOOK < len(items):
                        emit_S(*items[gidx + LOOK], gidx + LOOK)
                    emit_PV(*items[gidx], gidx)
                    for fn in sched.pop(gidx, []):
                        fn()
                for kk in sorted(sched):
                    for fn in sched[kk]:
                        fn()

            # ============ phase C: tail (norm rows + onstore) ============
            with tc.tile_pool(name="rowC", bufs=1) as rowc:
                rr0f = rowc.tile([128, NQ], F32R)
                nc.vector.memset(rr0f.bitcast(F32), 0.0)
                with tc.tile_pool(name="rowW", bufs=3) as roww:
                    sd = roww.tile([H, NQ], F32, tag="t")
                    nc.scalar.activation(
                        sd, argall, mybir.ActivationFunctionType.Sqrt)
                    rrt = roww.tile([H, NQ], F32, tag="t")
                    scr2 = roww.tile([H, NQ], F32, tag="t")
                    nc.vector.reciprocal_approx_accurate(
                        out=rrt, in_=sd, scratch=scr2)
                    nc.vector.tensor_copy(rr0f[0:H, :], rrt)

                # on = u * rr * weff -> onstore [128, CIN, NQ] bf16
                onstore = rowc.tile([128, CIN, NQ], BF16, tag="onstore")
                with (
                    tc.tile_pool(name="scrN", bufs=4) as scrp,
                    tc.tile_pool(name="psN", bufs=2, space="PSUM") as psN,
                ):
                    for h in range(H):
                        for qb in range(4):
                            sl = slice(qb * 512, (qb + 1) * 512)
                            rsl = slice((h % 2) * VD, (h % 2) * VD + VD)
                            rrbc = psN.tile([128, 512], F32, tag="rrbc")
                            nc.tensor.matmul(
                                rrbc, selfat[:, h, :], rr0f[:, sl],
                                start=True, stop=True)
                            on = scrp.tile([128, 512], F32, tag="on")
                            nc.vector.tensor_mul(
                                on[rsl, :], ustore[rsl, h // 2, sl],
                                rrbc[rsl, :])
                            nc.vector.tensor_scalar_mul(
                                onstore[rsl, h // 2, sl], on[rsl, :],
                                weff_t[rsl, :])

                # ============ phase D: proj (bf16, K=128) ============
                with (
                    tc.tile_pool(name="wp_p", bufs=1) as wpp,
                    tc.tile_pool(name="yd_p", bufs=3) as ydp,
                    tc.tile_pool(name="psY", bufs=2, space="PSUM") as psY,
                ):
                    wpb = wpp.tile([128, CIN, DIM], BF16)
                    nc.sync.dma_start(
                        out=wpb,
                        in_=wpbt[:, :].rearrange("v (c n) -> v c n", c=CIN))
                    bb = wpp.tile([128, DIM], F32)
                    nc.sync.dma_start(out=bb, in_=biasbc[:, :])
                    for qt in range(NQ // 128):
                        yps = psY.tile([128, 1024], F32, tag="y")
                        for sb in range(2):
                            for ci in range(CIN):
                                nc.tensor.matmul(
                                    yps[:, sb * 512:(sb + 1) * 512],
                                    onstore[:, ci, qt * 128:(qt + 1) * 128],
                                    wpb[:, ci, sb * 512:(sb + 1) * 512],
                                    start=(ci == 0),
                                    stop=(ci == CIN - 1),
                                )
                        yd = ydp.tile([128, 1024], F32, tag="yd")
                        nc.vector.tensor_add(yd, yps, bb)
                        nc.sync.dma_start(
                            out=y[qt * 128:(qt + 1) * 128, :], in_=yd)
    nc.finalize()
    return nc


def _make_inputs(x, w_qkv, w_proj, b_proj, sub_norm_w):
    wqkvt = np.ascontiguousarray(
        np.asarray(w_qkv, np.float32).T).astype(ml_dtypes.bfloat16)
    wprojt = np.ascontiguousarray(np.asarray(w_proj, np.float32).T)  # [c, out]
    # proj weights: partition (h%2)*64+vd, col (h//2)*DIM+out
    wpbt = np.ascontiguousarray(
        wprojt.reshape(CIN, 2, VD, DIM).transpose(1, 2, 0, 3)
        .reshape(128, CIN * DIM)).astype(ml_dtypes.bfloat16)
    biasbc = np.ascontiguousarray(
        np.tile(np.asarray(b_proj, np.float32).reshape(1, DIM), (128, 1)))
    # selfat[:, h, :]: [128,128]; row h one-hot -> cols (h%2)*64..+64
    selp = np.zeros((128, H, 128), np.float32)
    for h in range(H):
        po = (h % 2) * VD
        selp[h, h, po:po + VD] = 1.0
    selp = np.ascontiguousarray(selp.reshape(128, H * 128))
    weff = np.tile(
        (np.asarray(sub_norm_w, np.float32)
         * (1.0 - LAMBDA_INIT)).reshape(VD, 1), (2, 1))
    weff = np.ascontiguousarray(weff)
    return wqkvt, wpbt, biasbc, weff, selp


def _in_maps(inputs):
    x = np.asarray(inputs["x"], np.float32)
    wqkvt, wpbt, biasbc, weff, selp = _make_inputs(
        x, inputs["w_qkv"], inputs["w_proj"], inputs["b_proj"],
        inputs["sub_norm_w"])
    in_maps = []
    for c in range(NCORES):
        b, half = c // 2, c % 2
        xt = np.asarray(x[b].T)  # [DIM, N]
        if half == 1:  # query rows first
            xt = np.concatenate([xt[:, NQ:], xt[:, :NQ]], axis=1)
        in_maps.append({
            "xbt": np.ascontiguousarray(xt),
            "wqkvt": wqkvt,
            "wpbt": wpbt,
            "biasbc": biasbc,
            "weff": weff,
            "selp": selp,
        })
    return in_maps


def kernel(x, w_qkv, w_proj, b_proj, lambda_q1, lambda_k1, lambda_q2,
           lambda_k2, sub_norm_w):
    lam = float(
        np.exp(np.sum(np.float64(lambda_q1) * np.float64(lambda_k1)))
        - np.exp(np.sum(np.float64(lambda_q2) * np.float64(lambda_k2)))
        + LAMBDA_INIT
    )

    key = round(lam, 12)
    if key not in _CACHE:
        _CACHE[key] = build_nc(lam)
    nc = _CACHE[key]

    in_maps = _in_maps(dict(
        x=x, w_qkv=w_qkv, w_proj=w_proj, b_proj=b_proj, sub_norm_w=sub_norm_w))
    res = run_bass_kernel_spmd(nc, in_maps, list(range(NCORES)))
    out = np.empty((B, N, DIM), np.float32)
    for c in range(NCORES):
        b, half = c // 2, c % 2
        out[b, half * NQ:(half + 1) * NQ, :] = res.results[c]["y"]
    return out


# revision 21
# speedup vs baseline: 1.2852x; 1.1045x over previous
"""DiffAttention TRN2 kernel v3: 8-way (batch x seq-half) sharded, zero collectives.

v3 pipeline (ACT-exp roofline ~2.15ms/core; v2 was PE-bound at 3.03ms):
  - All matmul operands bf16 (fp32 runs 2-pass fp32_mode=HIGH at ~2x cost).
  - Phase A: qkv projections -> DRAM scratch (Q^T, K^T bf16, V bf16).
  - Phase B pair-centric: pair = (head h, branches 0/1). Q^T/K^T 128-row
    blocks naturally hold 4 combos in 4 32-partition strips; S matmuls are
    K=32 row-tiled (tile_position=(32s,0)) so strip MMs run concurrently.
    3-slot groups -> S into 3 PSUM banks -> one [128,1536] exp ACT (bf16
    out), ping-pong 3+3 banks; PV accumulates per-combo [65,512] (65th
    row = ones col of V_aug -> softmax denominators) in 2 more banks.
    Combine (u = o1 - lam*z1/z2*o2, arg = mean(u^2)+eps*z1^2) uses DVE rows
    + GPSIMD partition_broadcast/all_reduce; u kept in SBUF (ustore).
  - Phase C: rsqrt rows + sel-matmul broadcast -> onstore bf16.
  - Phase D: proj bf16, K=128 per chunk, bias via host-tiled broadcast.
"""

import sys

import numpy as np

for p in ("/opt/trn_rl_repo",):
    if p not in sys.path:
        sys.path.insert(0, p)

import ml_dtypes

import concourse.bass as bass
import concourse.bacc as bacc_mod
import concourse.bass_isa as bass_isa
import concourse.mybir as mybir
from concourse.bass_utils import run_bass_kernel_spmd
from concourse.tile import TileContext

F32 = mybir.dt.float32
F32R = mybir.dt.float32r
BF16 = mybir.dt.bfloat16

B, N, DIM, H, HD = 4, 4096, 1024, 16, 32
VD = 2 * HD  # 64, per-head v dim
NQ = 2048  # query rows per core
KT = N // 128  # 32 key tiles
CIN = DIM // 128  # 8 contraction tiles
NCORES = 8
LAMBDA_INIT = 0.2
EPS = 1e-5
SCALE = HD ** -0.5

_CACHE = {}


def build_nc(lam: float):
    nc = bacc_mod.Bacc(None, target_bir_lowering=False)

    xbt = nc.declare_dram_parameter("xbt", [DIM, N], F32, isOutput=False)
    wqkvt = nc.declare_dram_parameter("wqkvt", [DIM, 3 * DIM], BF16,
                                      isOutput=False)
    wpbt = nc.declare_dram_parameter("wpbt", [128, CIN * DIM], BF16,
                                     isOutput=False)
    biasbc = nc.declare_dram_parameter("biasbc", [128, DIM], F32,
                                       isOutput=False)
    weff = nc.declare_dram_parameter("weff", [128, 1], F32, isOutput=False)
    selp = nc.declare_dram_parameter("selp", [128, H * 128], F32,
                                     isOutput=False)
    y = nc.declare_dram_parameter("y", [NQ, DIM], F32, isOutput=True)

    qt_s = nc.dram_tensor("qt_scratch", [DIM, NQ], BF16)
    kt_s = nc.dram_tensor("kt_scratch", [DIM, N], BF16)
    v_s = nc.dram_tensor("v_scratch", [N, DIM], BF16)

    with nc.allow_low_precision(reason="bf16 matmul operands; bf16 stores"), \
         TileContext(nc) as tc:
        with (
            tc.tile_pool(name="const", bufs=1) as constp,
            tc.tile_pool(name="store", bufs=1) as storep,
        ):
            # selfat[:, h, :]: [128, 128] one-hot padded broadcast matrices
            selfat = constp.tile([128, H, 128], F32R)
            nc.sync.dma_start(
                out=selfat,
                in_=selp[:, :].rearrange("p (h v) -> p h v", v=128)
                .bitcast(F32R))
            weff_t = constp.tile([128, 1], F32)
            nc.sync.dma_start(out=weff_t, in_=weff[:, :])

            # persistent stores
            argall = storep.tile([H, NQ], F32)
            ustore = storep.tile([128, CIN, NQ], BF16)

            # ================= phase A: qkv (bf16) =================
            with (
                tc.tile_pool(name="xbt_p", bufs=2) as xbtp,
                tc.tile_pool(name="wq_p", bufs=1) as wqp,
                tc.tile_pool(name="drain_p", bufs=3) as drp,
                tc.tile_pool(name="psA", bufs=3, space="PSUM") as psA,
                tc.tile_pool(name="psAv", bufs=2, space="PSUM") as psAv,
            ):
                # preload all qkv weights once (per-ci DMAs stall the MMs)
                w_all = wqp.tile([128, CIN, 3 * DIM], BF16)
                nc.sync.dma_start(
                    out=w_all,
                    in_=wqkvt[:, :].rearrange("(c p) n -> p c n", p=128),
                )
                for tq in range(4):  # token quarters of 1024
                    xbf = xbtp.tile([128, CIN, 1024], F32, tag="xbf")
                    nc.sync.dma_start(
                        out=xbf,
                        in_=xbt[:, tq * 1024:(tq + 1) * 1024]
                        .rearrange("(t p) n -> p t n", p=128),
                    )
                    xb = xbtp.tile([128, CIN, 1024], BF16, tag="xb")
                    nc.vector.tensor_copy(xb, xbf)
                    for co in range(2 * CIN):  # 0..7 Q, 8..15 K
                        is_q = co < CIN
                        if is_q and tq >= 2:
                            continue
                        ps = psA.tile([128, 1024], F32, tag="ps")
                        for ci in range(CIN):
                            for sb in range(2):
                                nc.tensor.matmul(
                                    ps[:, sb * 512:(sb + 1) * 512],
                                    w_all[:, ci,
                                          co * 128:(co + 1) * 128],
                                    xb[:, ci, sb * 512:(sb + 1) * 512],
                                    start=(ci == 0),
                                    stop=(ci == CIN - 1),
                                )
                        dr = drp.tile([128, 1024], BF16, tag="dr")
                        nc.vector.tensor_copy(dr, ps)
                        dst = qt_s if is_q else kt_s
                        coo = co if is_q else co - CIN
                        nc.sync.dma_start(
                            out=dst[coo * 128:(coo + 1) * 128,
                                    tq * 1024:(tq + 1) * 1024],
                            in_=dr,
                        )
                    for cc in range(DIM // 512):
                        for kt in range(8):
                            psv = psAv.tile([128, 512], F32, tag="psv")
                            for ci in range(CIN):
                                nc.tensor.matmul(
                                    psv,
                                    xb[:, ci, kt * 128:(kt + 1) * 128],
                                    w_all[:, ci, 2 * DIM + cc * 512:
                                          2 * DIM + (cc + 1) * 512],
                                    start=(ci == 0),
                                    stop=(ci == CIN - 1),
                                )
                            drv = drp.tile([128, 512], BF16, tag="drv")
                            if kt % 2 == 0:
                                nc.vector.tensor_copy(drv, psv)
                            else:
                                nc.scalar.activation(
                                    drv, psv,
                                    mybir.ActivationFunctionType.Copy)
                            nc.sync.dma_start(
                                out=v_s[tq * 1024 + kt * 128:
                                        tq * 1024 + (kt + 1) * 128,
                                        cc * 512:(cc + 1) * 512],
                                in_=drv,
                            )

            # ================= phase B: attention =================
            # slots per (head h, qb): slot i = (br=i%2, kt=i//2), strip i%4
            # (K/Q strips duplicated at partitions 64-127 so all 3 slots of a
            #  group hit distinct 32-row PE groups -> fully concurrent S)
            slots_hq = [(i % 2, i // 2, i % 4) for i in range(2 * KT)]
            GPQ = (len(slots_hq) + 2) // 3  # 22 groups (last has 1 slot)
            groups_hq = [slots_hq[3 * i:3 * i + 3] for i in range(GPQ)]
            # groups whose exp runs on DVE (Schraudolph bf16) instead of ACT
            OFFLOAD_GS = (1, 5, 9, 13, 17)
            A16 = 128.0 / np.log(2.0)
            B16 = 16250.375

            with (
                tc.tile_pool(name="kf_p", bufs=2) as kfp,
                tc.tile_pool(name="qp_p", bufs=2) as qpp,
                tc.tile_pool(name="vh_p", bufs=2) as vhp,
                tc.tile_pool(name="exp_p", bufs=4) as expp,
                tc.tile_pool(name="cp_p", bufs=3) as cpp,
                tc.tile_pool(name="row_p", bufs=3) as rowp,
                tc.tile_pool(name="psS", bufs=2, space="PSUM") as psS,
                tc.tile_pool(name="psO", bufs=2, space="PSUM") as psO,
            ):
                st = {}
                sched = {}

                def emit_setup_h(h):
                    # K^T/Q^T 64-row head block duplicated to both halves
                    kf = kfp.tile([128, KT, 128], BF16, tag="kf")
                    qp = qpp.tile([128, NQ], BF16, tag="qp")
                    for half in range(2):
                        nc.sync.dma_start(
                            out=kf[64 * half:64 * half + 64, :, :],
                            in_=kt_s[h * 64:(h + 1) * 64, :]
                            .rearrange("p (k t) -> p k t", t=128),
                        )
                        nc.sync.dma_start(
                            out=qp[64 * half:64 * half + 64, :],
                            in_=qt_s[h * 64:(h + 1) * 64, :])
                    vh = vhp.tile([128, KT, 65], BF16, tag="vh")
                    nc.sync.dma_start(
                        out=vh[:, :, 0:VD],
                        in_=v_s[:, h * VD:(h + 1) * VD]
                        .rearrange("(k p) v -> p k v", p=128),
                    )
                    nc.vector.memset(vh[:, :, VD:65], 1.0)
                    st[("kf", h)], st[("qp", h)], st[("vh", h)] = kf, qp, vh
                    for key in ("kf", "qp", "vh"):
                        st.pop((key, h - 2), None)

                def emit_S(h, qb, g, gidx):
                    if g == 0 and qb == 0:
                        emit_setup_h(h)
                    kf, qp = st[("kf", h)], st[("qp", h)]
                    sps = psS.tile([128, 3, 512], F32, tag="s")
                    for j, (br, kt, s) in enumerate(groups_hq[g]):
                        nc.tensor.matmul(
                            sps[:, j, :],
                            kf[32 * s:32 * s + 32, kt, :],
                            qp[32 * s:32 * s + 32,
                               qb * 512:(qb + 1) * 512],
                            start=True, stop=True,
                            tile_position=(32 * s, 0),
                        )
                    n = len(groups_hq[g])
                    if g in OFFLOAD_GS:
                        ei = expp.tile([128, 3, 512], mybir.dt.int16,
                                       tag="e", name="ei")
                        nc.vector.tensor_scalar(
                            out=ei[:, 0:n, :], in0=sps[:, 0:n, :],
                            scalar1=float(A16 * SCALE), scalar2=float(B16),
                            op0=mybir.AluOpType.mult,
                            op1=mybir.AluOpType.add)
                        st[("ex", gidx)] = ei.bitcast(BF16)
                    else:
                        ex = expp.tile([128, 3, 512], BF16, tag="e",
                                       name="ex")
                        nc.scalar.activation(
                            ex[:, 0:n, :], sps[:, 0:n, :],
                            mybir.ActivationFunctionType.Exp, scale=SCALE,
                        )
                        st[("ex", gidx)] = ex

                def emit_PV(h, qb, g, gidx):
                    vh = st[("vh", h)]
                    ex = st.pop(("ex", gidx))
                    for j, (br, kt, s) in enumerate(groups_hq[g]):
                        if kt == 0:
                            st[("o", br)] = psO.tile(
                                [65, 512], F32, tag="o", name=f"o{br}")
                        nc.tensor.matmul(
                            st[("o", br)],
                            vh[:, kt, 0:65],
                            ex[:, j, :],
                            start=(kt == 0),
                            stop=(kt == KT - 1),
                        )
                    if g == GPQ - 1:
                        emit_drain(h, qb, gidx)

                def emit_drain(h, qb, gidx):
                    # full-tile copies free the 2 PV banks ASAP
                    o1p = st.pop(("o", 0))
                    o2p = st.pop(("o", 1))
                    sl = slice(qb * 512, (qb + 1) * 512)
                    cp_o1 = cpp.tile([65, 512], F32, tag="co1", name="co1")
                    nc.vector.tensor_copy(cp_o1, o1p)
                    cp_o2 = cpp.tile([65, 512], F32, tag="co2", name="co2")
                    nc.vector.tensor_copy(cp_o2, o2p)
                    # z rows to partition 0 (DVE is lane-locked; DMA moves)
                    z1 = rowp.tile([1, 512], F32, tag="z1", name="z1")
                    nc.sync.dma_start(out=z1, in_=cp_o1[VD:65, :])
                    z2 = rowp.tile([1, 512], F32, tag="z2", name="z2")
                    nc.sync.dma_start(out=z2, in_=cp_o2[VD:65, :])

                    def part1(h=h, sl=sl, cp_o1=cp_o1, cp_o2=cp_o2,
                              z1=z1, z2=z2):
                        rzs = rowp.tile([1, 512], F32, tag="rzs", name="rzs")
                        rz2 = rowp.tile([1, 512], F32, tag="rz2", name="rz2")
                        nc.vector.reciprocal_approx_accurate(
                            out=rz2, in_=z2, scratch=rzs)
                        trow = rowp.tile([1, 512], F32, tag="tr", name="tr")
                        nc.vector.scalar_tensor_tensor(
                            out=trow, in0=z1, scalar=float(lam), in1=rz2,
                            op0=mybir.AluOpType.mult,
                            op1=mybir.AluOpType.mult)
                        tbc = cpp.tile([VD, 512], F32, tag="tbc", name="tbc")
                        nc.gpsimd.partition_broadcast(tbc, trow)
                        st[("t", h, sl.start)] = (tbc, cp_o1, cp_o2, z1)

                    def part2(h=h, qb=qb, sl=sl):
                        tbc, cp_o1, cp_o2, z1 = st.pop(("t", h, sl.start))
                        x1 = cpp.tile([VD, 512], F32, tag="x1", name="x1")
                        nc.vector.tensor_mul(x1, cp_o2[0:VD, :], tbc)
                        u_t = cpp.tile([VD, 512], BF16, tag="ut", name="ut")
                        nc.vector.tensor_sub(u_t, cp_o1[0:VD, :], x1)
                        rsl = slice((h % 2) * VD, (h % 2) * VD + VD)
                        nc.sync.dma_start(
                            out=ustore[rsl, h // 2, sl], in_=u_t)
                        u2 = cpp.tile([VD, 512], F32, tag="u2", name="u2")
                        nc.vector.tensor_mul(u2, u_t, u_t)
                        s2 = cpp.tile([VD, 512], F32, tag="s2", name="s2")
                        nc.gpsimd.partition_all_reduce(
                            s2, u2, channels=VD,
                            reduce_op=bass_isa.ReduceOp.add)
                        st[("u", h, sl.start)] = (s2, z1)

                    def part3(h=h, sl=sl):
                        s2, z1 = st.pop(("u", h, sl.start))
                        ze = rowp.tile([1, 512], F32, tag="ze", name="ze")
                        nc.vector.tensor_scalar_mul(
                            ze, z1, float(EPS ** 0.5))
                        zsq = rowp.tile([1, 512], F32, tag="zq", name="zq")
                        nc.vector.tensor_mul(zsq, ze, ze)
                        arg0 = rowp.tile([1, 512], F32, tag="ar", name="ar")
                        nc.vector.scalar_tensor_tensor(
                            out=arg0, in0=s2[0:1, :],
                            scalar=1.0 / VD, in1=zsq,
                            op0=mybir.AluOpType.mult,
                            op1=mybir.AluOpType.add)
                        nc.sync.dma_start(out=argall[h:h + 1, sl], in_=arg0)

                    sched.setdefault(gidx + 2, []).append(part1)
                    sched.setdefault(gidx + 5, []).append(part2)
                    sched.setdefault(gidx + 8, []).append(part3)

                items = [(h, qb, g)
                         for h in range(H)
                         for qb in range(4)
                         for g in range(GPQ)]
                LOOK = 2
                for j in range(LOOK):
                    emit_S(*items[j], j)
                for gidx in range(len(items)):
                    if gidx + LOOK < len(items):
                        emit_S(*items[gidx + LOOK], gidx + LOOK)
                    emit_PV(*items[gidx], gidx)
                    for fn in sched.pop(gidx, []):
                        fn()
                for kk in sorted(sched):
                    for fn in sched[kk]:
                        fn()

            # ============ phase C: tail (norm rows + onstore) ============
            with tc.tile_pool(name="rowC", bufs=1) as rowc:
                rr0f = rowc.tile([128, NQ], F32R)
                nc.vector.memset(rr0f.bitcast(F32), 0.0)
                with tc.tile_pool(name="rowW", bufs=3) as roww:
                    sd = roww.tile([H, NQ], F32, tag="t")
                    nc.scalar.activation(
                        sd, argall, mybir.ActivationFunctionType.Sqrt)
                    rrt = roww.tile([H, NQ], F32, tag="t")
                    scr2 = roww.tile([H, NQ], F32, tag="t")
                    nc.vector.reciprocal_approx_accurate(
                        out=rrt, in_=sd, scratch=scr2)
                    nc.vector.tensor_copy(rr0f[0:H, :], rrt)

                # on = u * rr * weff -> onstore, then proj per qb block
                onstore = rowc.tile([128, CIN, NQ], BF16, tag="onstore")
                with (
                    tc.tile_pool(name="scrN", bufs=4) as scrp,
                    tc.tile_pool(name="wp_p", bufs=1) as wpp,
                    tc.tile_pool(name="yd_p", bufs=3) as ydp,
                    tc.tile_pool(name="psN", bufs=2, space="PSUM") as psN,
                    tc.tile_pool(name="psY", bufs=2, space="PSUM") as psY,
                ):
                    wpb = wpp.tile([128, CIN, DIM], BF16)
                    nc.sync.dma_start(
                        out=wpb,
                        in_=wpbt[:, :].rearrange("v (c n) -> v c n", c=CIN))
                    bb = wpp.tile([128, DIM], F32)
                    nc.sync.dma_start(out=bb, in_=biasbc[:, :])
                    for qb in range(4):
                        sl = slice(qb * 512, (qb + 1) * 512)
                        for h in range(H):
                            rsl = slice((h % 2) * VD, (h % 2) * VD + VD)
                            rrbc = psN.tile([128, 512], F32, tag="rrbc")
                            nc.tensor.matmul(
                                rrbc, selfat[:, h, :], rr0f[:, sl],
                                start=True, stop=True)
                            on = scrp.tile([128, 512], F32, tag="on")
                            nc.vector.tensor_mul(
                                on[rsl, :], ustore[rsl, h // 2, sl],
                                rrbc[rsl, :])
                            nc.vector.tensor_scalar_mul(
                                onstore[rsl, h // 2, sl], on[rsl, :],
                                weff_t[rsl, :])
                        for qt in range(qb * 4, qb * 4 + 4):
                            yps = psY.tile([128, 1024], F32, tag="y")
                            for sb in range(2):
                                for ci in range(CIN):
                                    nc.tensor.matmul(
                                        yps[:, sb * 512:(sb + 1) * 512],
                                        onstore[:, ci,
                                                qt * 128:(qt + 1) * 128],
                                        wpb[:, ci,
                                            sb * 512:(sb + 1) * 512],
                                        start=(ci == 0),
                                        stop=(ci == CIN - 1),
                                    )
                            yd = ydp.tile([128, 1024], F32, tag="yd")
                            nc.vector.tensor_add(yd, yps, bb)
                            nc.sync.dma_start(
                                out=y[qt * 128:(qt + 1) * 128, :], in_=yd)
    nc.finalize()
    return nc


def _make_inputs(x, w_qkv, w_proj, b_proj, sub_norm_w):
    wqkvt = np.ascontiguousarray(
        np.asarray(w_qkv, np.float32).T).astype(ml_dtypes.bfloat16)
    wprojt = np.ascontiguousarray(np.asarray(w_proj, np.float32).T)  # [c, out]
    # proj weights: partition (h%2)*64+vd, col (h//2)*DIM+out
    wpbt = np.ascontiguousarray(
        wprojt.reshape(CIN, 2, VD, DIM).transpose(1, 2, 0, 3)
        .reshape(128, CIN * DIM)).astype(ml_dtypes.bfloat16)
    biasbc = np.ascontiguousarray(
        np.tile(np.asarray(b_proj, np.float32).reshape(1, DIM), (128, 1)))
    # selfat[:, h, :]: [128,128]; row h one-hot -> cols (h%2)*64..+64
    selp = np.zeros((128, H, 128), np.float32)
    for h in range(H):
        po = (h % 2) * VD
        selp[h, h, po:po + VD] = 1.0
    selp = np.ascontiguousarray(selp.reshape(128, H * 128))
    weff = np.tile(
        (np.asarray(sub_norm_w, np.float32)
         * (1.0 - LAMBDA_INIT)).reshape(VD, 1), (2, 1))
    weff = np.ascontiguousarray(weff)
    return wqkvt, wpbt, biasbc, weff, selp


def _in_maps(inputs):
    x = np.asarray(inputs["x"], np.float32)
    wqkvt, wpbt, biasbc, weff, selp = _make_inputs(
        x, inputs["w_qkv"], inputs["w_proj"], inputs["b_proj"],
        inputs["sub_norm_w"])
    in_maps = []
    for c in range(NCORES):
        b, half = c // 2, c % 2
        xt = np.asarray(x[b].T)  # [DIM, N]
        if half == 1:  # query rows first
            xt = np.concatenate([xt[:, NQ:], xt[:, :NQ]], axis=1)
        in_maps.append({
            "xbt": np.ascontiguousarray(xt),
            "wqkvt": wqkvt,
            "wpbt": wpbt,
            "biasbc": biasbc,
            "weff": weff,
            "selp": selp,
        })
    return in_maps


def kernel(x, w_qkv, w_proj, b_proj, lambda_q1, lambda_k1, lambda_q2,
           lambda_k2, sub_norm_w):
    lam = float(
        np.exp(np.sum(np.float64(lambda_q1) * np.float64(lambda_k1)))
        - np.exp(np.sum(np.float64(lambda_q2) * np.float64(lambda_k2)))
        + LAMBDA_INIT
    )

    key = round(lam, 12)
    if key not in _CACHE:
        _CACHE[key] = build_nc(lam)
    nc = _CACHE[key]

    in_maps = _in_maps(dict(
        x=x, w_qkv=w_qkv, w_proj=w_proj, b_proj=b_proj, sub_norm_w=sub_norm_w))
    res = run_bass_kernel_spmd(nc, in_maps, list(range(NCORES)))
    out = np.empty((B, N, DIM), np.float32)
    for c in range(NCORES):
        b, half = c // 2, c % 2
        out[b, half * NQ:(half + 1) * NQ, :] = res.results[c]["y"]
    return out
